# revision 10
# baseline (speedup 1.0000x reference)
"""MoNet layer Trainium2 kernel (data-parallel over batch on 8 NeuronCores).

Math (per batch b, node i, neighbor j, gaussian k):
  edge      = ~isnan(coord[b,i,j,0])
  rho/theta = coord channels (0 where non-edge in reference; here NaN->sentinel)
  a_k       = mu_rho[k]  (upstream bug: theta also uses mu_rho)
  cr_k      = 0.5/(1e-14+sig_rho[k]^2),  ct_k = 0.5/(1e-14+sig_theta[k]^2)
  ang       = min(d, |2pi-d|), d = |theta - a_k|
  w[b,i,j,k]= edge * exp(-cr_k (rho-a_k)^2 - ct_k ang^2)
  agg[b,i,k,f] = sum_j w[b,i,j,k] x[b,j,f]
  out[b,i,:]   = (agg.reshape(K*F) @ fc_W.T + fc_b) * mask[b,i]

Key identities used on-device:
  ct*ang^2 = (sqrt(ct)*|theta - a + pi| - sqrt(ct)*pi)^2   (valid: theta-a+pi in [-2pi,2pi])
  non-edges: rho := 1e4 (NaN dropped by DVE min) => exp arg ~ -1e7 => w = 0 exactly
  fc fused:  out[i,o] = sum_{j,k} w[j,(i)k] z[j,(k,o)],  z = x @ W_k^T per k

Layouts: w tiles [j=128, (b,i)=1024] per (jchunk,k); z [j=128,(k,o)=1600] per (b,jc);
out accumulated in PSUM as out^T [o=64, i=256] per b, transposed back via PE.
"""

import numpy as np

import concourse.bass as bass
import concourse.mybir as mybir
import concourse.tile as tile
from concourse.bass_utils import run_bass_kernel_spmd

mdt = mybir.dt
F32 = mdt.float32
F32R = mdt.float32r
I32 = mdt.int32
ALU = mybir.AluOpType
AF = mybir.ActivationFunctionType

B, N, K, F_IN, F_OUT = 32, 256, 25, 64, 64
NCORES = 8
BL = B // NCORES            # batches per core
BI = BL * N                 # flattened (b, i) free dim = 1024
PI = np.pi


def _split_excess_waits(nc, max_waits=1):
    """This walrus build rejects instructions carrying more than one sync
    wait. Hoist extra waits onto NoOp instructions inserted just before the
    over-subscribed instruction (same engine => program order preserves
    semantics)."""
    ctr = 0
    for f in nc.m.functions:
        for bb in f.blocks:
            changed = False
            new = []
            for inst in bb.instructions:
                si = inst.sync_info
                if si is not None and si.on_wait and len(si.on_wait) > max_waits:
                    waits = list(si.on_wait)
                    extra, keep = waits[:-max_waits], waits[-max_waits:]
                    for i in range(0, len(extra), max_waits):
                        nop = mybir.InstNoOp(name=nc.get_next_instruction_name())
                        ctr += 1
                        nop.engine = inst.engine
                        nop.sync_info = mybir.SyncInfo(
                            on_wait=extra[i:i + max_waits], on_update=[])
                        nc.register_instruction(nop)
                        new.append(nop)
                    inst.sync_info = mybir.SyncInfo(
                        on_wait=keep, on_update=list(si.on_update))
                    changed = True
                new.append(inst)
            if changed:
                bb.instructions = new


def _f(v):
    return float(np.float32(v))


def build_program(consts):
    """Build the per-core Bass program. consts: dict of per-k host scalars."""
    sa_t, ba_t, two_a, neg_cr = (
        consts["sa_t"], consts["ba_t"], consts["two_a"], consts["neg_cr"]
    )
    nc = bass.Bass("TRN2", target_bir_lowering=False, debug=False)

    xs_ap = nc.dram_tensor("xs", [BL, N, F_IN], F32, kind="ExternalInput").ap()
    coord_ap = nc.dram_tensor("coord", [BL, N, N, 2], F32, kind="ExternalInput").ap()
    maskr_ap = nc.dram_tensor("maskr", [BL, F_OUT, N], F32, kind="ExternalInput").ap()
    fcw_ap = nc.dram_tensor("fcW", [F_OUT, K * F_IN], F32, kind="ExternalInput").ap()
    fcb_ap = nc.dram_tensor("fcb", [F_OUT, 1], F32, kind="ExternalInput").ap()
    ident_ap = nc.dram_tensor("ident", [128, 128], F32, kind="ExternalInput").ap()
    ktab_ap = nc.dram_tensor("ktab", [128, 3 * K], F32, kind="ExternalInput").ap()
    out_ap = nc.dram_tensor("out", [BL, N, F_OUT], F32, kind="ExternalOutput").ap()

    with tile.TileContext(nc) as tc:
        import contextlib

        with contextlib.ExitStack() as ctx:
            persist = ctx.enter_context(tc.tile_pool(name="persist", bufs=1))
            coordp = ctx.enter_context(tc.tile_pool(name="coordp", bufs=4))
            trps = ctx.enter_context(tc.tile_pool(name="trps", bufs=2, space="PSUM"))
            zps = ctx.enter_context(tc.tile_pool(name="zps", bufs=2, space="PSUM"))
            outps = ctx.enter_context(tc.tile_pool(name="outps", bufs=1, space="PSUM"))
            work = ctx.enter_context(tc.tile_pool(name="work", bufs=2))
            epi = ctx.enter_context(tc.tile_pool(name="epi", bufs=2))

            # ---- small constants in ----
            ident = persist.tile([128, 128], F32, tag="ident")
            nc.sync.dma_start(ident[:], ident_ap[:])
            ktab = persist.tile([128, 3 * K], F32, tag="ktab")
            nc.sync.dma_start(ktab[:], ktab_ap[:])
            fcb = persist.tile([F_OUT, 1], F32, tag="fcb")
            nc.sync.dma_start(fcb[:], fcb_ap[:])
            fcw = persist.tile([F_OUT, K * F_IN], F32, tag="fcw")
            nc.sync.dma_start(fcw[:], fcw_ap[:])
            masks = persist.tile([F_OUT, BL * N], F32, tag="masks")
            for b in range(BL):
                nc.sync.dma_start(masks[:, b * N:(b + 1) * N], maskr_ap[b])

            # ---- fcWT[f, (k,o)] = fc_W[o, k*F+f] via 25 PE transposes ----
            fcwt = persist.tile([F_IN, K * F_OUT], F32R, tag="fcwt")
            for k in range(K):
                tp = trps.tile([F_IN, F_OUT], F32, tag="trp")
                nc.tensor.transpose(tp[:], fcw[:, k * F_IN:(k + 1) * F_IN],
                                    ident[:F_OUT, :F_OUT])
                nc.scalar.copy(fcwt[:, k * F_OUT:(k + 1) * F_OUT], tp[:])

            # ---- x^T per b: xT[f=64, j=256] ----
            xts = []
            for b in range(BL):
                xt = persist.tile([F_IN, N], F32R, tag=f"xt{b}")
                for jc in range(2):
                    xsb = coordp.tile([128, F_IN], F32, tag="xin")
                    nc.sync.dma_start(xsb[:], xs_ap[b, jc * 128:(jc + 1) * 128])
                    tp = trps.tile([F_IN, 128], F32, tag="trp")
                    nc.tensor.transpose(tp[:], xsb[:], ident[:])
                    nc.scalar.copy(xt[:, jc * 128:(jc + 1) * 128], tp[:])
                xts.append(xt)

            # ---- coord -> rhoT/thetaT [j=128, (b,i)=1024] per jchunk ----
            # free index layout: jc*BI + b*N + i   (BI = BL*N = 1024)
            rt = persist.tile([128, 2 * BI], F32, tag="rt")
            tt = persist.tile([128, 2 * BI], F32, tag="tt")
            for b in range(BL):
                for ic in range(2):
                    csb = coordp.tile([128, 2 * N], F32, tag="coord")
                    nc.sync.dma_start(csb[:], coord_ap[b, ic * 128:(ic + 1) * 128])
                    for jc in range(2):
                        for ch, dst in ((0, rt), (1, tt)):
                            tp = trps.tile([128, 128], F32, tag="trp")
                            tsrc = csb[:, 2 * jc * 128 + ch: 2 * (jc + 1) * 128: 2]
                            nc.tensor.transpose(tp[:], tsrc, ident[:])
                            nc.vector.tensor_copy(
                                dst[:, jc * BI + b * N + ic * 128:
                                    jc * BI + b * N + (ic + 1) * 128],
                                tp[:])
            # NaN cleanup (DVE min drops NaN): rho->1e4 (kills edge via exp),
            # theta->10 (harmless finite)
            nc.vector.tensor_scalar_min(rt[:], rt[:], 1.0e4)
            nc.vector.tensor_scalar_min(tt[:], tt[:], 10.0)
            p2 = persist.tile([128, 2 * BI], F32, tag="p2")
            nc.vector.tensor_tensor(p2[:], rt[:], rt[:], ALU.mult)

            # ---- phase A: z[b,jc][j=128, (k,o)=1600] = x^T chunk @ fcWT ----
            KO = K * F_OUT
            zg = [0, 512, 1024, 1536, KO]  # k-group free slices
            zsb = []
            for b in range(BL):
                zb = []
                for jc in range(2):
                    z = persist.tile([128, KO], F32R, tag=f"z{b}{jc}")
                    for g in range(4):
                        lo, hi = zg[g], zg[g + 1]
                        zp = zps.tile([128, 512], F32, tag="zp")
                        nc.tensor.matmul(
                            zp[:, : hi - lo],
                            xts[b][:, jc * 128:(jc + 1) * 128],
                            fcwt[:, lo:hi],
                            start=True, stop=True)
                        nc.vector.tensor_copy(z[:, lo:hi], zp[:, : hi - lo])
                    zb.append(z)
                zsb.append(zb)

            # ---- out^T accumulators [o=64, i=256] per b ----
            outp = [outps.tile([F_OUT, N], F32, tag=f"op{b}", name=f"op{b}")
                    for b in range(BL)]

            # ---- phase B: gaussian weights + accumulation ----
            # M_SPLIT of the K tiles compute |.| via gpsimd-affine + DVE
            # bitwise-and instead of ACT Abs, to balance engine load.
            M_SPLIT = 8
            for k in range(K):
                u = work.tile([128, 2 * BI], F32, tag="u")
                if k < M_SPLIT:
                    y = work.tile([128, 2 * BI], F32, tag="y")
                    nc.gpsimd.tensor_scalar(
                        y[:], tt[:], sa_t[k], ba_t[k], ALU.mult, ALU.add)
                    nc.vector.tensor_scalar(
                        u[:].bitcast(I32), y[:].bitcast(I32),
                        0x7FFFFFFF, None, ALU.bitwise_and)
                else:
                    nc.scalar.activation(u[:], tt[:], AF.Abs,
                                         bias=ktab[:, 3 * k:3 * k + 1],
                                         scale=sa_t[k])
                t = work.tile([128, 2 * BI], F32, tag="t")
                nc.scalar.activation(t[:], u[:], AF.Square,
                                     bias=ktab[:, 3 * k + 1:3 * k + 2], scale=1.0)
                xx = work.tile([128, 2 * BI], F32, tag="xx")
                nc.vector.scalar_tensor_tensor(
                    xx[:], rt[:], two_a[k], p2[:], ALU.mult, ALU.subtract)
                nc.vector.scalar_tensor_tensor(
                    t[:], xx[:], neg_cr[k], t[:], ALU.mult, ALU.add)
                w = work.tile([128, 2 * BI], F32R, tag="w")
                nc.scalar.activation(w[:], t[:], AF.Exp,
                                     bias=ktab[:, 3 * k + 2:3 * k + 3],
                                     scale=-1.0)
                for b in range(BL):
                    for jc in range(2):
                        nc.tensor.matmul(
                            outp[b][:],
                            zsb[b][jc][:, k * F_OUT:(k + 1) * F_OUT],
                            w[:, jc * BI + b * N: jc * BI + (b + 1) * N],
                            start=(k == 0 and jc == 0),
                            stop=(k == K - 1 and jc == 1))

            # ---- epilogue: bias + mask, transpose back, store ----
            for b in range(BL):
                ot = epi.tile([F_OUT, N], F32, tag="ot")
                nc.vector.scalar_tensor_tensor(
                    ot[:], outp[b][:], fcb[:, 0:1], masks[:, b * N:(b + 1) * N],
                    ALU.add, ALU.mult)
                for ih in range(2):
                    tp = trps.tile([128, F_OUT], F32, tag="trp")
                    nc.tensor.transpose(
                        tp[:], ot[:, ih * 128:(ih + 1) * 128],
                        ident[:F_OUT, :F_OUT])
                    osb = epi.tile([128, F_OUT], F32, tag="osb")
                    nc.scalar.copy(osb[:], tp[:])
                    nc.sync.dma_start(out_ap[b, ih * 128:(ih + 1) * 128], osb[:])

    _split_excess_waits(nc)
    return nc


def _host_consts(coords_mu, sigma_rho, sigma_theta):
    a = np.asarray(coords_mu, np.float64)[0]            # [K] (bug: mu_rho everywhere)
    sr = np.asarray(sigma_rho, np.float64)
    st = np.asarray(sigma_theta, np.float64)
    cr = 0.5 / (1e-14 + sr * sr)
    ct = 0.5 / (1e-14 + st * st)
    sct = np.sqrt(ct)
    consts = {
        "sa_t": [_f(v) for v in sct],                   # y = sa_t*theta + ba_t
        "ba_t": [_f(v) for v in sct * (PI - a)],
        "two_a": [_f(v) for v in 2.0 * a],              # X = 2a*rho - rho^2
        "neg_cr": [_f(v) for v in -cr],                 # s = -cr*X + T
    }
    ktab = np.zeros((128, 3 * K), np.float32)
    ktab[:, 0::3] = (sct * (PI - a)).astype(np.float32)  # U = Abs(sa_t*th + ba_t)
    ktab[:, 1::3] = -(sct * PI).astype(np.float32)       # T = (U - sqrt(ct)*pi)^2
    ktab[:, 2::3] = -(cr * a * a).astype(np.float32)     # exp bias
    return consts, ktab


_CACHE = {}


def kernel(**inputs):
    x = np.ascontiguousarray(np.asarray(inputs["x"], np.float32))
    coord = np.ascontiguousarray(np.asarray(inputs["coord"], np.float32))
    mask = np.asarray(inputs["mask"], np.float32)
    coords_mu = np.asarray(inputs["coords_mu"], np.float32)
    sigma_rho = np.asarray(inputs["sigma_rho"], np.float32)
    sigma_theta = np.asarray(inputs["sigma_theta"], np.float32)
    fc_W = np.ascontiguousarray(np.asarray(inputs["fc_W"], np.float32))
    fc_b = np.asarray(inputs["fc_b"], np.float32)

    consts, ktab = _host_consts(coords_mu, sigma_rho, sigma_theta)

    key = (tuple(consts["sa_t"]), tuple(consts["ba_t"]),
           tuple(consts["two_a"]), tuple(consts["neg_cr"]))
    if key not in _CACHE:
        _CACHE.clear()
        _CACHE[key] = build_program(consts)
    nc = _CACHE[key]

    ident = np.eye(128, dtype=np.float32)
    fcb = np.ascontiguousarray(fc_b.reshape(F_OUT, 1))
    in_maps = []
    for c in range(NCORES):
        sl = slice(c * BL, (c + 1) * BL)
        maskr = np.ascontiguousarray(
            np.broadcast_to(mask[sl][:, None, :], (BL, F_OUT, N)).astype(np.float32))
        in_maps.append({
            "xs": x[sl], "coord": coord[sl], "maskr": maskr,
            "fcW": fc_W, "fcb": fcb, "ident": ident, "ktab": ktab,
        })

    res = run_bass_kernel_spmd(nc, in_maps, core_ids=list(range(NCORES)))
    out = np.concatenate([res.results[c]["out"] for c in range(NCORES)], axis=0)
    return out.astype(np.float32)


